# revision 16
# baseline (speedup 1.0000x reference)
"""Trainium2 Bass kernel for nn_MultiHeadAttention_3590592660317.

Sequence-sharded across 8 cores (512 query rows each); each core
redundantly computes full K/V (cheap) plus its own Q rows, attention,
and output rows -> no collectives; the host concatenates the slices.

v2 design: fp8 attention weights via DoubleRow PV.
  - exp outputs fp8e4 (max-shifted); PV runs in DoubleRow perf mode:
    one matmul per (pair, head, kg) contracts 256 keys at 0.5 cyc/row,
    quartering PV's PE time vs bf16.  Tolerance check: fp8 p+v sim
    rel_err 4.6e-3 vs the 2e-2 gate.
  - fp8 exp overflows to inf at ~248 (no saturation), so exp args are
    shifted by a per-query max estimate M (max over the first 256 keys,
    measured gap to the true max 1.96 -> p_max ~7, 35x headroom).  The
    shift rides in the QK matmul as a 33rd contraction row: kT row 32
    holds 1.0 (host-DMA'd bf16 ones), qT row 32 holds -max(q.k sample).
    Cost of a matmul is output-free-size only, so the extra row is free.
  - kT/qT tiles hold 2 heads each at 33-row strips (offsets 0/64);
    4 tiles = 8 heads.  Per (pair,kg,sub,head): one QK matmul into a
    1-bank [128,512] PSUM tile; fine tiles × 4 bufs keep the ACT exp
    stream fed across pair switches.  PV accumulators: [48,1024] slices
    of 2-bank tiles (DoubleRow requires dst partition 0); v carries a
    ones column so row 32 accumulates the softmax sums.
  - M sample pass in prep: 32 [q,k]-oriented matmuls over 256 keys,
    free-axis max reduces (DVE/gpsimd), one [128,128] xbar transpose,
    8 tiny DMAs scatter the per-head rows into qT row 32/96.
  - Two streaming phases: pairs {0,1} (windows built JIT under it),
    then pairs {2,3} (g1 kT windows + wo/bo load + partial out-proj
    for heads 0-3 ride in the ACT slack).  PV for the first 2 rows of
    phase B is deferred until the phase-A accumulators drain (PSUM).
  - Norms: sums from pvt row 32; 1/s via reciprocal (DVE) mid-stream
    for pairs 0/1 and exp(-ln s) on the idle ACT at the tail for 2/3.
"""

import sys

if "/opt/trn_rl_repo" not in sys.path:
    sys.path.insert(0, "/opt/trn_rl_repo")

import numpy as np

import concourse.bass as bass
import concourse.tile as tile
from concourse import mybir
from concourse.bass_utils import run_bass_kernel_spmd
from concourse.tile import add_dep_helper

F32 = mybir.dt.float32
BF16 = mybir.dt.bfloat16
F8 = mybir.dt.float8e4
I32 = mybir.dt.int32
I16 = mybir.dt.int16

N = 4096
IN_DIM = 256
H = 8
HD = 32
DIN = 259
NC = 8
R = N // NC  # 512 rows per core

_KDIMS = (128, 128, 4)  # contraction chunks: 256 feats + (3 coords + ones)
SCALE = 1.0 / float(np.sqrt(HD))


def _split_oversized_waits(nc, max_waits=1):
    """This walrus build only encodes one sync-wait per instruction; move
    excess on_wait conditions onto preceding same-engine NOPs."""
    nsplit = 0
    for f in nc.m.functions:
        for b in f.blocks:
            new_instrs = []
            for ins in b.instructions:
                si = getattr(ins, "sync_info", None)
                waits = list(si.on_wait) if si is not None and si.on_wait else []
                changed = False
                while len(waits) > max_waits:
                    chunk, waits = waits[:max_waits], waits[max_waits:]
                    nop = mybir.InstNoOp(name=f"{ins.name}-ws{nsplit}", ins=[], outs=[])
                    nop.engine = ins.engine
                    nop.sync_info = mybir.SyncInfo(on_wait=chunk, on_update=[])
                    new_instrs.append(nop)
                    nsplit += 1
                    changed = True
                if changed:
                    ins.sync_info = mybir.SyncInfo(
                        on_wait=waits, on_update=list(si.on_update or [])
                    )
                new_instrs.append(ins)
            b.instructions = new_instrs
    return nsplit


def build_nc(split=True):
    nc = bass.Bass()

    x = nc.dram_tensor("x", [N, IN_DIM], F32, kind="ExternalInput")
    coordsT = nc.dram_tensor("coordsT", [3, N], I32, kind="ExternalInput")
    xq = nc.dram_tensor("xq", [R, IN_DIM], F32, kind="ExternalInput")
    cqT = nc.dram_tensor("cqT", [3, R], I32, kind="ExternalInput")
    oneskr = nc.dram_tensor("oneskr", [2, 4 * N], I16, kind="ExternalInput")
    wq = nc.dram_tensor("wq", [H, DIN, HD], F32, kind="ExternalInput")
    bq = nc.dram_tensor("bq", [H, HD], F32, kind="ExternalInput")
    wk = nc.dram_tensor("wk", [H, DIN, HD], F32, kind="ExternalInput")
    bk = nc.dram_tensor("bk", [H, HD], F32, kind="ExternalInput")
    wv = nc.dram_tensor("wv", [H, DIN, HD], F32, kind="ExternalInput")
    bv = nc.dram_tensor("bv", [H, HD], F32, kind="ExternalInput")
    wo = nc.dram_tensor("wo", [IN_DIM, IN_DIM], F32, kind="ExternalInput")
    bo = nc.dram_tensor("bo", [IN_DIM], F32, kind="ExternalInput")
    out = nc.dram_tensor("out", [R, IN_DIM], F32, kind="ExternalOutput")

    with tile.TileContext(nc) as tc:
        _body(tc, nc, x, coordsT, xq, cqT, oneskr,
              wq, bq, wk, bk, wv, bv, wo, bo, out)

    if split:
        _split_oversized_waits(nc)
    return nc


def _body(tc, nc, x, coordsT, xq, cqT, oneskr, wq, bq, wk, bk, wv, bv, wo, bo, out):
    from contextlib import ExitStack

    ctx = ExitStack()
    with ctx:
        sing = ctx.enter_context(tc.tile_pool(name="sing", bufs=1))
        stream = ctx.enter_context(tc.tile_pool(name="stream", bufs=2))
        ftp = ctx.enter_context(tc.tile_pool(name="ftp", bufs=4))
        wstg = ctx.enter_context(tc.tile_pool(name="wstg", bufs=2))
        ptp = ctx.enter_context(tc.tile_pool(name="ptp", bufs=14))
        psF = ctx.enter_context(tc.tile_pool(name="psF", bufs=3, space="PSUM"))
        psV = ctx.enter_context(tc.tile_pool(name="psV", bufs=1, space="PSUM"))

        # ---------------- persistent SBUF tiles ----------------
        # kT/qT: tile t holds head 2t at rows 0-31 (+ones/M row 32) and
        # head 2t+1 at rows 64-95 (+row 96).  33-row strips, offsets 0/64.
        kTbig = sing.tile([128, 4, N], BF16, tag="kTbig", name="kTbig")
        qTbig = sing.tile([128, 4, R], BF16, tag="qTbig", name="qTbig")
        xcT0 = sing.tile([128, N], BF16, tag="xcT0", name="xcT0")
        xcT1 = sing.tile([128, N], BF16, tag="xcT1", name="xcT1")
        cnT = sing.tile([4, N], BF16, tag="cnT", name="cnT")  # coords + ones
        xcq0 = sing.tile([128, R], BF16, tag="xcq0", name="xcq0")
        xcq1 = sing.tile([128, R], BF16, tag="xcq1", name="xcq1")
        cnq = sing.tile([4, R], BF16, tag="cnq", name="cnq")
        # v fp8: per key-chunk 8 heads x 34 (32 v + ones col + pad);
        # 272B chunk stride satisfies the dual-fp8 LDW 16B alignment rule
        v_sb = sing.tile([128, N // 128, 272], F8, tag="vv", name="vv")
        catT = [sing.tile([128, R], BF16, tag=f"catT{g}", name=f"catT{g}") for g in range(2)]
        x_all = sing.tile([128, 4, 4, IN_DIM], F32, tag="x_all", name="x_all")
        xq_all = sing.tile([128, 4, IN_DIM], F32, tag="xq_all", name="xq_all")
        ss_all = sing.tile([128, 36], F32, tag="ss_all", name="ss_all")
        inv_all = sing.tile([128, 36], F32, tag="inv_all", name="inv_all")
        cq_f_g = sing.tile([4, R], F32, tag="cq_f_g", name="cq_f_g")
        # M-estimate machinery
        colmax = sing.tile([128, 128], F32, tag="colmax", name="colmax")
        colmaxb = sing.tile([128, 128], BF16, tag="colmaxb", name="colmaxb")
        t1 = sing.tile([128, 128], BF16, tag="t1", name="t1")

        wk_sb = [sing.tile([_KDIMS[c], H, HD], BF16, tag=f"wk{c}", name=f"wk{c}") for c in range(3)]
        wq_sb = [sing.tile([_KDIMS[c], H, HD], BF16, tag=f"wq{c}", name=f"wq{c}") for c in range(3)]
        wv_sb = [sing.tile([_KDIMS[c], H * HD], BF16, tag=f"wv{c}", name=f"wv{c}") for c in range(3)]
        wo_sb = [sing.tile([128, IN_DIM], BF16, tag=f"wo{c}", name=f"wo{c}") for c in range(2)]
        bo_f32 = sing.tile([1, IN_DIM], F32, tag="bo32", name="bo32")
        ones_colf = sing.tile([1, 128], F32, tag="ones_colf", name="ones_colf")
        ones_rowb = sing.tile([1, HD], BF16, tag="ones_rowb", name="ones_rowb")
        ones_rowf = sing.tile([1, HD], F32, tag="ones_rowf", name="ones_rowf")
        bo_bc = sing.tile([128, IN_DIM], F32, tag="bo_bc", name="bo_bc")
        partial = sing.tile([128, 4, IN_DIM], F32, tag="partial", name="partial")

        nc.vector.memset(ones_colf, 1.0)
        nc.vector.memset(ones_rowb, 1.0)
        nc.vector.memset(ones_rowf, 1.0)
        nc.vector.memset(colmaxb, 0.0)
        v4 = v_sb.rearrange("p c (h z) -> p c h z", z=34)

        # ---------------- weight loads + casts ----------------
        def load_weights_main():
            nc.vector.memset(cq_f_g, 1.0)
            nc.gpsimd.dma_start(out=cq_f_g.bitcast(I32)[0:3], in_=cqT[:, :])
            cwide = wstg.tile([128, 128], F32, tag="cwide", name="cwide")
            nc.vector.memset(cwide[96:128, :], 1.0)
            nc.sync.dma_start(
                out=cwide.bitcast(I32)[0:96],
                in_=coordsT[:, :].rearrange("c (j f) -> (c j) f", f=128),
            )
            # ones rows of kT (row 32 and 96 across all 4 tiles)
            kt16 = kTbig.bitcast(I16)
            nc.gpsimd.dma_start(out=kt16[32:33, :, :], in_=oneskr[0:1, :])
            nc.gpsimd.dma_start(out=kt16[96:97, :, :], in_=oneskr[1:2, :])
            stgs = {}
            for c in (2, 0, 1):
                kd = (128, 128, 3)[c]
                dsl = slice(c * 128, c * 128 + kd)
                for wi, (w_dram, b_dram) in enumerate(((wk, bk), (wq, bq), (wv, bv))):
                    stg = sing.tile(
                        [128 if c != 2 else 4, H, HD], F32,
                        tag=f"stg{c}{wi}", name=f"stg{c}{wi}",
                    )
                    weng = (nc.gpsimd, nc.sync, nc.scalar)[c]
                    weng.dma_start(
                        out=stg[:kd], in_=w_dram[:, dsl, :].rearrange("h d k -> d h k")
                    )
                    if c == 2:
                        nc.gpsimd.dma_start(
                            out=stg[3:4],
                            in_=b_dram[:, :].rearrange("h k -> (h k)")[None, :]
                            .rearrange("a (h k) -> a h k", h=H),
                        )
                    stgs[(c, wi)] = stg
            nc.vector.tensor_copy(out=cwide[0:96], in_=cwide.bitcast(I32)[0:96])
            cwb = wstg.tile([128, 128], BF16, tag="cwb", name="cwb")
            nc.vector.tensor_scalar_min(out=cwb, in0=cwide, scalar1=100.0)
            nc.sync.dma_start(out=cnT, in_=cwb)
            nc.vector.tensor_copy(out=cq_f_g[0:3], in_=cq_f_g.bitcast(I32)[0:3])
            nc.vector.tensor_scalar_min(out=cnq, in0=cq_f_g, scalar1=100.0)
            for wi, w_tile in enumerate((wk_sb[2], wq_sb[2], wv_sb[2])):
                nc.vector.tensor_copy(out=w_tile[:4], in_=stgs[(2, wi)][:4])
            return stgs

        def load_weights_casts(stgs):
            for c in (0, 1):
                for wi, w_tile in enumerate((wk_sb[c], wq_sb[c], wv_sb[c])):
                    w3 = (
                        w_tile if w_tile.shape[1] == H
                        else w_tile.rearrange("d (h k) -> d h k", h=H)
                    )
                    nc.vector.tensor_copy(out=w3[:128], in_=stgs[(c, wi)][:128])
            nc.vector.memset(v4[:, :, :, 32:33], 1.0)

        def load_weights_tail():
            for c in range(2):
                stg = wstg.tile([128, IN_DIM], F32, tag="wstg2", name="wstg2")
                nc.gpsimd.dma_start(out=stg, in_=wo[c * 128:(c + 1) * 128, :])
                nc.vector.tensor_copy(out=wo_sb[c], in_=stg)
            nc.gpsimd.dma_start(out=bo_f32, in_=bo[None, :])
            bct = psF.tile([128, 1024], F32, tag="f", name="bo_bc_ps")
            nc.tensor.matmul(
                bct[:, 0:IN_DIM], lhsT=ones_colf, rhs=bo_f32, start=True, stop=True
            )
            nc.vector.tensor_copy(out=bo_bc, in_=bct[:, 0:IN_DIM])
            for rw in range(4):
                nc.vector.tensor_add(
                    out=xq_all[:, rw, :], in0=xq_all[:, rw, :], in1=bo_bc
                )

        # ---------------- norm machinery ----------------
        def dma_x_window(w, eng):
            eng.dma_start(
                out=x_all[:, w % 4],
                in_=x[w * 512:(w + 1) * 512, :].rearrange("(c p) d -> p c d", p=128),
            )

        def squares_act(xbuf, nchunk, ss_col):
            for i in range(nchunk):
                sq = stream.tile([128, IN_DIM], F32, tag="sqa", name="sqa")
                nc.scalar.activation(
                    out=sq, in_=xbuf[:, i, :],
                    func=mybir.ActivationFunctionType.Square,
                    accum_out=ss_all[:, ss_col + i:ss_col + i + 1],
                )

        def squares_gp(w):
            sq = stream.tile([128, 4, IN_DIM], F32, tag="sqg", name="sqg")
            nc.gpsimd.tensor_mul(out=sq, in0=x_all[:, w % 4], in1=x_all[:, w % 4])
            nc.vector.tensor_reduce(
                out=ss_all[:, 4 * w:4 * w + 4], in_=sq,
                axis=mybir.AxisListType.X, op=mybir.AluOpType.add,
            )

        def inv_lnexp(col0, ncols):
            lnb = stream.tile([128, 4], F32, tag="lnb", name="lnb")
            nc.scalar.activation(
                out=lnb, in_=ss_all[:, col0:col0 + ncols],
                func=mybir.ActivationFunctionType.Ln,
            )
            nc.scalar.activation(
                out=inv_all[:, col0:col0 + ncols], in_=lnb,
                func=mybir.ActivationFunctionType.Exp, scale=-0.5,
            )

        def scale_mul(rc, lo, hi, xap, ss_col):
            inv1 = inv_all[:, ss_col:ss_col + 1]
            nc.vector.tensor_scalar_mul(
                out=lo[:, rc % 4, :], in0=xap[:, 0:128], scalar1=inv1
            )
            nc.vector.tensor_scalar_mul(
                out=hi[:, rc % 4, :], in0=xap[:, 128:256], scalar1=inv1
            )

        def scale_transpose(nw, t0, t1_, lo, hi, eng0=None, eng1=None):
            nsl = slice(nw * 512, (nw + 1) * 512)
            (eng0 or nc.sync).dma_start_transpose(
                out=t0[:, nsl].rearrange("f (g r) -> f g r", r=128),
                in_=lo.rearrange("p g r -> p (g r)"),
            )
            (eng1 or nc.sync).dma_start_transpose(
                out=t1_[:, nsl].rearrange("f (g r) -> f g r", r=128),
                in_=hi.rearrange("p g r -> p (g r)"),
            )

        def xcT_chunk(c, full=True):
            if full:
                return (xcT0, xcT1, cnT)[c]
            return (xcq0, xcq1, cnq)[c]

        def window_chain(w, phase_even):
            """Half of window w's norm chain (split over 2 emission slots)."""
            if phase_even:
                if w + 1 <= 7:
                    dma_x_window(w + 1, nc.gpsimd)
                squares_gp(w)
                inv_lnexp(4 * w, 4)
                flo = ftp.tile([128, 4, 128], BF16, tag="flo", name="flo")
                fhi = ftp.tile([128, 4, 128], BF16, tag="fhi", name="fhi")
                scale_mul(0, flo, fhi, x_all[:, w % 4, 0, :], 4 * w + 0)
                scale_mul(1, flo, fhi, x_all[:, w % 4, 1, :], 4 * w + 1)
                return (flo, fhi)
            return None

        def window_chain2(w, fl):
            flo, fhi = fl
            scale_mul(2, flo, fhi, x_all[:, w % 4, 2, :], 4 * w + 2)
            scale_mul(3, flo, fhi, x_all[:, w % 4, 3, :], 4 * w + 3)
            scale_transpose(w, xcT0, xcT1, flo, fhi)

        # ---------------- projections ----------------
        def kq_copy(dstbig, ps, g, nsl):
            # dense 4-head PSUM -> 33-strip layout; 2 copies DVE, 2 gpsimd
            for j in range(4):
                t = 2 * g + j // 2
                off = 64 * (j % 2)
                nc.vector.tensor_copy(
                    out=dstbig[off:off + 32, t, nsl],
                    in_=ps[32 * j:32 * j + 32, :],
                )

        def kq_window(w_sb_, dstbig, g, nw, full=True):
            nsl = slice(nw * 512, (nw + 1) * 512) if full else slice(0, R)
            pstw = psF.tile([128, 1024], F32, tag="f", name="proj")
            pst = pstw[:, 0:512]
            for c in (2, 0, 1):
                kd = _KDIMS[c]
                nc.tensor.matmul(
                    pst,
                    lhsT=w_sb_[c][:kd, g * 4:g * 4 + 4, :].rearrange("d h k -> d (h k)"),
                    rhs=xcT_chunk(c, full)[:kd, nsl],
                    start=(c == 2),
                    stop=(c == 1),
                    skip_group_check=True,
                )
            kq_copy(dstbig, pst, g, nsl)

        def v_chunk2(rc):
            pstw = psF.tile([128, 1024], F32, tag="f", name="projv")
            pst = pstw[:, 0:512]
            for b in range(2):
                ps = pst[:, b * 256:b * 256 + H * HD]
                for c in (2, 0, 1):
                    kd = _KDIMS[c]
                    nc.tensor.matmul(
                        ps,
                        lhsT=xcT_chunk(c)[:kd, (rc + b) * 128:(rc + b + 1) * 128],
                        rhs=wv_sb[c][:kd, :],
                        start=(c == 2),
                        stop=(c == 1),
                        skip_group_check=True,
                    )
            nc.vector.tensor_copy(
                out=v4[:, rc:rc + 2, :, 0:HD],
                in_=pst.rearrange("p (b x) -> p b x", b=2)
                .rearrange("p b (h k) -> p b h k", h=H),
            )

        # ---------------- attention ----------------
        class _St:
            pass

        def attn_begin(pair):
            st = _St()
            st.pair = pair
            pvt = psV.tile([128, 1024], F32, tag="pv", name="pv")
            st.pvps = pvt
            st.prev = None  # (kg, [pt_a0, pt_a1])
            st.pend = []    # deferred (kg, pts)
            st.qk_last = None
            st.pv_last = None
            return st

        def qk_block(st, kg):
            """4 QK matmuls (pos-alternating) + 2 coarse exps; returns pts."""
            p = st.pair
            atts = [psF.tile([128, 1024], F32, tag="f", name="att") for _ in range(2)]
            pts = [ptp.tile([128, 1024], F8, tag="pt", name="pt") for _ in range(2)]
            for sub in range(2):
                kc = 2 * kg + sub
                for a in range(2):
                    off = 64 * a
                    mm = nc.tensor.matmul(
                        atts[a][:, sub * 512:(sub + 1) * 512],
                        lhsT=kTbig[off:off + 33, p, kc * 128:(kc + 1) * 128],
                        rhs=qTbig[off:off + 33, p, :],
                        start=True, stop=True,
                        tile_position=(off, 0),
                    )
                    if st.pv_last is not None:
                        add_dep_helper(mm.ins, st.pv_last.ins, sync=False)
                    st.qk_last = mm
                    if sub == 1:
                        nc.scalar.activation(
                            out=pts[a], in_=atts[a],
                            func=mybir.ActivationFunctionType.Exp, scale=SCALE,
                        )
            return pts

        def pv_block(st, kg, pts, first, last, dr):
            for a in range(2):
                h = 2 * st.pair + a
                if dr:
                    mm = nc.tensor.matmul(
                        st.pvps[0:33, a * 512:(a + 1) * 512],
                        lhsT=v4[:, 2 * kg:2 * kg + 2, h, 0:33],
                        rhs=pts[a].rearrange("p (t q) -> p t q", t=2),
                        start=first, stop=last,
                        perf_mode=mybir.MatmulPerfMode.DoubleRow,
                        tile_position=(0, 0),
                        skip_group_check=True,
                    )
                    if st.qk_last is not None:
                        add_dep_helper(mm.ins, st.qk_last.ins, sync=False)
                    st.pv_last = mm
                else:
                    for sub in range(2):
                        mm = nc.tensor.matmul(
                            st.pvps[0:33, a * 512:(a + 1) * 512],
                            lhsT=v4[:, 2 * kg + sub, h, 0:33],
                            rhs=pts[a][:, sub * 512:(sub + 1) * 512],
                            start=(first and sub == 0),
                            stop=(last and sub == 1),
                            tile_position=(0, 0),
                            skip_group_check=True,
                        )
                        if st.qk_last is not None:
                            add_dep_helper(mm.ins, st.qk_last.ins, sync=False)
                        st.pv_last = mm

        def attn_row(st, kg, defer=False, dr=False):
            """Emit QK/exp for kg and the lagged PV for kg-1."""
            pts = qk_block(st, kg)
            if st.prev is not None:
                pkg, ppts = st.prev
                if defer:
                    st.pend.append((pkg, ppts, dr))
                else:
                    pv_block(st, pkg, ppts, first=(pkg == 0), last=False, dr=dr)
            st.prev = (kg, pts)

        def drain_pend(st, nbatch=1):
            for _ in range(nbatch):
                if st.pend:
                    pkg, ppts, dr = st.pend.pop(0)
                    pv_block(st, pkg, ppts, first=(pkg == 0), last=False, dr=dr)

        def attn_flush(st, dr=False):
            drain_pend(st, len(st.pend))
            pkg, ppts = st.prev
            pv_block(st, pkg, ppts, first=(pkg == 0), last=True, dr=dr)
            st.prev = None

        def attn_norm(st, g, hpos, act_recip=False):
            """catT[g] rows [64*hpos, 64*hpos+64) <- pvt normalized."""
            s_sb = stream.tile([1, 1024], F32, tag="s_sb", name="s_sb")
            nc.vector.tensor_copy(out=s_sb, in_=st.pvps[32:33, :])
            if act_recip:
                lnr = stream.tile([1, 1024], F32, tag="lnr", name="lnr")
                nc.scalar.activation(
                    out=lnr, in_=s_sb, func=mybir.ActivationFunctionType.Ln
                )
                r_sb = stream.tile([1, 1024], BF16, tag="r_sbb", name="r_sbb")
                nc.scalar.activation(
                    out=r_sb, in_=lnr,
                    func=mybir.ActivationFunctionType.Exp, scale=-1.0,
                )
            else:
                r_sb = stream.tile([1, 1024], F32, tag="r_sbf", name="r_sbf")
                nc.vector.reciprocal(out=r_sb, in_=s_sb)
            ones_lhs = ones_rowb if act_recip else ones_rowf
            rbct = psF.tile([128, 1024], F32, tag="f", name="rbc")
            for a in range(2):
                nc.tensor.matmul(
                    rbct[32 * a:32 * a + 32, 0:512],
                    lhsT=ones_lhs,
                    rhs=r_sb[:, a * 512:(a + 1) * 512],
                    start=True, stop=True,
                    skip_group_check=True,
                    tile_position=(0, 32 * a),
                )
            rbc_sb = stream.tile([64, 512], F32, tag="rbc_sb", name="rbc_sb")
            nc.vector.tensor_copy(out=rbc_sb, in_=rbct[0:64, 0:512])
            for a in range(2):
                nc.vector.tensor_mul(
                    out=catT[g][64 * hpos + 32 * a:64 * hpos + 32 * a + 32, :],
                    in0=st.pvps[0:32, a * 512:(a + 1) * 512],
                    in1=rbc_sb[32 * a:32 * a + 32, :],
                )

        # ---------------- emission: prep ----------------
        for rc in range(4):
            eng = (nc.sync, nc.scalar)[rc % 2]
            eng.dma_start(out=xq_all[:, rc, :], in_=xq[rc * 128:(rc + 1) * 128, :])
        for rc in range(4):
            eng = (nc.sync, nc.scalar, nc.gpsimd)[rc % 3]
            eng.dma_start(out=x_all[:, 0, rc, :], in_=x[rc * 128:(rc + 1) * 128, :])
        stgs = load_weights_main()

        squares_act(xq_all, 4, 32)
        inv_lnexp(32, 4)
        squares_act(x_all[:, 0], 4, 0)
        inv_lnexp(0, 4)

        flo = ftp.tile([128, 4, 128], BF16, tag="flo", name="flo")
        fhi = ftp.tile([128, 4, 128], BF16, tag="fhi", name="fhi")
        for rc in range(4):
            scale_mul(rc, flo, fhi, xq_all[:, rc, :], 32 + rc)
        scale_transpose(0, xcq0, xcq1, flo, fhi, eng0=nc.sync, eng1=nc.sync)
        flo = ftp.tile([128, 4, 128], BF16, tag="flo", name="flo")
        fhi = ftp.tile([128, 4, 128], BF16, tag="fhi", name="fhi")
        for rc in range(4):
            scale_mul(rc, flo, fhi, x_all[:, 0, rc, :], rc)
        scale_transpose(0, xcT0, xcT1, flo, fhi, eng0=nc.scalar, eng1=nc.sync)
        load_weights_casts(stgs)

        # q projection (both groups) + kT window 0
        for g in range(2):
            kq_window(wq_sb, qTbig, g, 0, full=False)
        dma_x_window(1, nc.gpsimd)
        kq_window(wk_sb, kTbig, 0, 0)
        kq_window(wk_sb, kTbig, 1, 0)
        v_chunk2(0)
        v_chunk2(2)

        # ----- M sample pass: per-head max over first 128 keys -----
        # 8 sample outputs packed per coarse PSUM tile -> 1 wide reduce each
        for t in range(4):
            sps = psF.tile([128, 1024], F32, tag="f", name="sample")
            for j in range(8):
                h = 2 * t + j // 4
                qb = j % 4
                off = 64 * (h % 2)
                nc.tensor.matmul(
                    sps[:, j * 128:(j + 1) * 128],
                    lhsT=qTbig[off:off + 32, t, qb * 128:(qb + 1) * 128],
                    rhs=kTbig[off:off + 32, t, 0:128],
                    start=True, stop=True,
                    tile_position=(off, 0),
                )
            nc.vector.tensor_reduce(
                out=colmax[:, 8 * t:8 * t + 8],
                in_=sps.rearrange("p (j f) -> p j f", f=128),
                axis=mybir.AxisListType.X, op=mybir.AluOpType.max,
            )
            if t == 1:
                fl = window_chain(1, True)
                window_chain2(1, fl)
        nc.vector.tensor_scalar_mul(out=colmaxb[:, 0:32], in0=colmax[:, 0:32], scalar1=-1.0)
        nc.sync.dma_start_transpose(out=t1, in_=colmaxb)
        for h in range(H):
            t, off = h // 2, 64 * (h % 2)
            nc.sync.dma_start(
                out=qTbig[32 + off:33 + off, t, :].rearrange("a (q j) -> a q j", j=128),
                in_=t1[4 * h:4 * h + 4, 0:128],
            )
        kq_window(wk_sb, kTbig, 0, 1)
        v_chunk2(4)
        v_chunk2(6)
        # windows 2-3 fully built in prep (PE/DVE idle-ish here)
        for w in (2, 3):
            dma_x_window(w + 1, nc.gpsimd)
            squares_act(x_all[:, w % 4], 4, 4 * w)
            inv_lnexp(4 * w, 4)
            flo = ftp.tile([128, 4, 128], BF16, tag="flo", name="flo")
            fhi = ftp.tile([128, 4, 128], BF16, tag="fhi", name="fhi")
            for rc in range(4):
                scale_mul(rc, flo, fhi, x_all[:, w % 4, rc, :], 4 * w + rc)
            scale_transpose(w, xcT0, xcT1, flo, fhi)
            kq_window(wk_sb, kTbig, 0, w)
            v_chunk2(4 * w)
            v_chunk2(4 * w + 2)

        # ----- 4 single-pair phases; PE kept just-saturated for pstate -----
        # dr=True halves a PV block's PE cost; used where builds load the PE.
        def phase(pair, prev_st, prev_pos, dr_sel, extra):
            if prev_st is not None:
                attn_flush(prev_st, dr=True)
            st = attn_begin(pair)
            for r in range(16):
                if r == 2:
                    drain_pend(st, 2)
                attn_row(st, r, defer=(r < 2 and pair > 0),
                         dr=dr_sel(r))
                if prev_st is not None and r == 0:
                    attn_norm(prev_st, prev_pos[0], prev_pos[1], act_recip=True)
                extra(r)
            return st

        # phase 0: pair 0; builds g0 kq/v for w4..w7 (w0-3 from prep)
        def extra0(r):
            if 2 <= r <= 9:
                w = r // 2 + 3
                if r % 2 == 0:
                    extra0.fl = window_chain(w, True)
                else:
                    window_chain2(w, extra0.fl)
            if r % 2 == 0:
                wb = r // 2 + 2
                if 4 <= wb <= 7:
                    kq_window(wk_sb, kTbig, 0, wb)
            else:
                wb = (r - 1) // 2 + 2
                if 4 <= wb <= 7:
                    v_chunk2(4 * wb)
                    v_chunk2(4 * wb + 2)

        st0 = phase(0, None, None, lambda r: True, extra0)

        # phase 1: pair 1; builds g1 kq w1..w7
        def extra1(r):
            if r % 2 == 0 and r <= 13:
                wb = r // 2 + 1
                if wb <= 7:
                    kq_window(wk_sb, kTbig, 1, wb)
            if r == 14:
                load_weights_tail()

        st1 = phase(1, st0, (0, 0), lambda r: True, extra1)

        # phase 2: pair 2; partial out-proj for heads 0-3
        def extra2(r):
            if r % 2 == 1 and 3 <= r <= 9:
                rw = (r - 3) // 2
                pst = psF.tile([128, 1024], F32, tag="f", name="outp0")
                nc.tensor.matmul(
                    pst[:, 0:IN_DIM],
                    lhsT=catT[0][:, rw * 128:(rw + 1) * 128],
                    rhs=wo_sb[0],
                    start=True, stop=True,
                    skip_group_check=True,
                )
                nc.vector.tensor_add(
                    out=partial[:, rw, :], in0=pst[:, 0:IN_DIM], in1=xq_all[:, rw, :]
                )

        st2 = phase(2, st1, (0, 1), lambda r: True, extra2)

        # phase 3: pair 3; fold pair-2's out-proj half into partial
        def extra3(r):
            if r % 2 == 1 and 5 <= r <= 11:
                rw = (r - 5) // 2
                pst = psF.tile([128, 1024], F32, tag="f", name="outp2")
                nc.tensor.matmul(
                    pst[:, 0:IN_DIM],
                    lhsT=catT[1][0:64, rw * 128:(rw + 1) * 128],
                    rhs=wo_sb[1][0:64, :],
                    start=True, stop=True,
                    skip_group_check=True,
                )
                nc.vector.tensor_add(
                    out=partial[:, rw, :], in0=pst[:, 0:IN_DIM], in1=partial[:, rw, :]
                )

        st3 = phase(3, st2, (1, 0), lambda r: True, extra3)

        attn_flush(st3)
        attn_norm(st3, 1, 1, act_recip=True)

        # ---------------- tail: out = catT[1][64:]^T wo[1][64:] + partial ----
        for rw in range(4):
            rsl = slice(rw * 128, (rw + 1) * 128)
            pst = psF.tile([128, 1024], F32, tag="f", name="outp1")
            nc.tensor.matmul(
                pst[:, 0:IN_DIM],
                lhsT=catT[1][64:128, rsl],
                rhs=wo_sb[1][64:128, :],
                start=True, stop=True,
                skip_group_check=True,
            )
            o_sb = x_all[:, 0, rw, :]
            nc.vector.tensor_add(out=o_sb, in0=pst[:, 0:IN_DIM], in1=partial[:, rw, :])
            nc.sync.dma_start(out=out[rsl, :], in_=o_sb)


_NC_CACHE = None


def _get_nc():
    global _NC_CACHE
    if _NC_CACHE is None:
        _NC_CACHE = build_nc()
    return _NC_CACHE


def kernel(_trace=False, **inputs):
    trace = _trace
    x = np.ascontiguousarray(np.asarray(inputs["x"], dtype=np.float32))
    coords = np.asarray(inputs["coords"], dtype=np.int32)
    coordsT = np.ascontiguousarray(coords.T)

    common = {
        "x": x,
        "coordsT": coordsT,
        "oneskr": np.full((2, 4 * N), 16256, dtype=np.int16),  # bf16 1.0
        "wq": np.ascontiguousarray(np.asarray(inputs["wq"], np.float32)),
        "bq": np.ascontiguousarray(np.asarray(inputs["bq"], np.float32)),
        "wk": np.ascontiguousarray(np.asarray(inputs["wk"], np.float32)),
        "bk": np.ascontiguousarray(np.asarray(inputs["bk"], np.float32)),
        "wv": np.ascontiguousarray(np.asarray(inputs["wv"], np.float32)),
        "bv": np.ascontiguousarray(np.asarray(inputs["bv"], np.float32)),
        "wo": np.ascontiguousarray(np.asarray(inputs["wo"], np.float32)),
        "bo": np.ascontiguousarray(np.asarray(inputs["bo"], np.float32)),
    }
    in_maps = []
    for c in range(NC):
        rsl = slice(c * R, (c + 1) * R)
        m = dict(common)
        m["xq"] = np.ascontiguousarray(x[rsl])
        m["cqT"] = np.ascontiguousarray(coordsT[:, rsl])
        in_maps.append(m)

    nc = _get_nc()
    res = run_bass_kernel_spmd(nc, in_maps, list(range(NC)), trace=trace)
    out = np.concatenate([res.results[c]["out"] for c in range(NC)], axis=0)
    if trace:
        return out, res
    return out
